# revision 23
# baseline (speedup 1.0000x reference)
"""Multi-head causal attention (B=2, T=2048, C=1024, H=16) on 8 Trainium2
NeuronCores, tensor-parallel over heads (2 heads per core).

v1 rewrite of the working baseline, targeting sustained Tensor-engine
occupancy (HAM stays un-throttled) and reduced ScalarE/DVE overhead:

  - software-pipelined emission: phase-1 (QKV proj) of chunk i+1 and
    phase-4 (out proj) of chunk i-1 are interleaved as "filler" PE work
    inside chunk i's attention k-loop; the last AV_DELAY AV matmuls and
    the softmax normalization are deferred past the chunk boundary so
    the PE never waits on the exp pipeline tail.
  - exp batched 1024-wide: both heads' score tiles land in one
    [128, 1024] PSUM tile (2 banks) and one ACTIVATE handles them.
  - out-projection contracts 128 (both heads stacked) instead of 2x64.
  - phase-1 bias-adds run on DVE (tensor_scalar add); ScalarE does
    (almost) nothing but exp.
  - x kept fully SBUF-resident (8 x [128, 4096] bf16), output written
    bf16, mask tiles bf16, V path bf16 end-to-end.

Host-side layouts are pre-tiled for contiguous DMA rows; the 8 partial
[C, BT] outputs are summed (and bias added) on the host.
"""

import os
import sys
from collections import deque

for _p in ("/opt/trn_rl_repo", "/root/.axon_site/_ro/trn_rl_repo"):
    if os.path.isdir(_p) and _p not in sys.path:
        sys.path.insert(0, _p)

import ml_dtypes
import numpy as np

import concourse.bacc as bacc
import concourse.bass as bass
import concourse.mybir as mybir
import concourse.tile as tile
from concourse.bass_utils import run_bass_kernel_spmd
from concourse.masks import make_identity

B, T, C, H, D = 2, 2048, 1024, 16, 64
NCORES = 8
BT = B * T                      # 4096 flattened tokens
TC = 512                        # token chunk (matmul free dim)
NTC = BT // TC                  # 8 token chunks
FP = mybir.dt.float32
FPR = mybir.dt.float32r
BF = mybir.dt.bfloat16
ACT = mybir.ActivationFunctionType
NEG = -1.0e9
AV_DELAY = 2                    # k-tiles the AV matmul trails the scores

LAST_RESULTS = None             # stashed BassKernelResults for test harness


def build_nc():
    nc = bacc.Bacc(None, target_bir_lowering=False, debug=False)

    xt = nc.declare_dram_parameter("xt", [C, BT], BF, isOutput=False)
    wc = nc.declare_dram_parameter("wc", [128, 3072], BF, isOutput=False)
    wout = nc.declare_dram_parameter("wout", [128, C], BF, isOutput=False)
    bqkv = nc.declare_dram_parameter("bqkv", [128, 3], FP, isOutput=False)
    masks = nc.declare_dram_parameter("masks", [128, 4096], BF, isOutput=False)
    onesr = nc.declare_dram_parameter("onesr", [1, 64], FP, isOutput=False)
    # yt[i*128 + p, m*512 + t] = y_partial[m*128 + p, i*512 + t]
    yt = nc.declare_dram_parameter("yt", [C, BT], BF, isOutput=True)

    with tile.TileContext(nc) as tc:
        with (
            tc.tile_pool(name="const", bufs=1) as cpool,
            tc.tile_pool(name="big", bufs=1) as bigpool,
            tc.tile_pool(name="sb", bufs=2) as sbpool,
            tc.tile_pool(name="ps", bufs=2, space="PSUM") as pspool,
        ):
            # ---- constants; DMA order = startup order ----
            # wc split per k-slice so the first phase-1 matmul only waits
            # on a 96 KB transfer; chunk-0 x columns land first.
            wc_sb = cpool.tile([128, 3072], BF)
            xks = []
            for k in range(8):
                xk = cpool.tile([128, BT], BF, name=f"xk{k}")
                xks.append(xk)
            for k in range(8):
                nc.sync.dma_start(out=wc_sb[:, k * 384:(k + 1) * 384],
                                  in_=wc[:, k * 384:(k + 1) * 384])
                nc.sync.dma_start(out=xks[k][:, 0:TC],
                                  in_=xt[k * 128:(k + 1) * 128, 0:TC])
            bq_sb = cpool.tile([128, 3], FP)
            nc.sync.dma_start(out=bq_sb[:], in_=bqkv[:, :])
            for k in range(8):
                nc.sync.dma_start(out=xks[k][:, TC:BT],
                                  in_=xt[k * 128:(k + 1) * 128, TC:BT])
            wout_sb = cpool.tile([128, C], BF)
            nc.sync.dma_start(out=wout_sb[:], in_=wout[:, :])
            masks_sb = cpool.tile([128, 4096], BF)
            nc.sync.dma_start(out=masks_sb[:], in_=masks[:, :])
            onesr_sb = cpool.tile([1, 64], FPR)
            nc.sync.dma_start(out=onesr_sb[:], in_=onesr.bitcast(FPR)[:, :])
            ident = cpool.tile([128, 128], BF)
            make_identity(nc, ident)

            # ---- HAM warm-up: keep the PE busy on scratch data while the
            # initial DMAs land, so real work starts at 2.4 GHz ----
            dummy = cpool.tile([128, TC], BF, name="dummy")
            nc.vector.memset(dummy[:], 0.0)
            for _ in range(72):
                dps = pspool.tile([128, TC], FP, tag="m", bufs=2, name="dps")
                nc.tensor.matmul(dps[:], dummy[:, 0:128], dummy[:],
                                 start=True, stop=True)

            # ---- persistent intermediates ----
            QT = bigpool.tile([128, BT], BF)
            KT = bigpool.tile([128, BT], BF)
            # V in [token, dim] layout, 130 cols per 128-token block:
            # [V_h0 (64) | ones | V_h1 (64) | ones]; memset once puts the
            # ones columns in place, transposed V overwrites the rest.
            vaug = bigpool.tile([128, 32 * 130], BF)
            nc.vector.memset(vaug[:], 1.0)

            vts = {}
            p1state = {}

            def make_phase1_units(i):
                """QKV projection + V transpose for chunk i, split into
                ~2-matmul granules for fine filler interleaving."""
                t0 = i * TC

                def g_granule(g, k0):
                    def emit():
                        if k0 == 0:
                            p1state[(i, g)] = pspool.tile(
                                [128, TC], FP, tag="m", bufs=2,
                                name=f"p1q{i}g{g}")
                        ps = p1state[(i, g)]
                        for k in (k0, k0 + 1):
                            nc.tensor.matmul(
                                ps[:],
                                wc_sb[:, k * 384 + g * 128:
                                      k * 384 + (g + 1) * 128],
                                xks[k][:, t0:t0 + TC],
                                start=(k == 0), stop=(k == 7),
                            )
                        if k0 == 6:
                            del p1state[(i, g)]
                            if g < 2:
                                dest = (QT, KT)[g][:, t0:t0 + TC]
                            else:
                                vt = sbpool.tile([128, TC], BF, tag="vt",
                                                 bufs=2, name=f"vt{i}")
                                vts[i] = vt
                                dest = vt[:]
                            nc.vector.tensor_scalar_add(
                                out=dest, in0=ps[:],
                                scalar1=bq_sb[:, g:g + 1])
                    return emit

                def t_granule(j0):
                    def emit():
                        vt = vts[i]
                        for j in (j0, j0 + 1):
                            jj = i * 4 + j
                            tp = pspool.tile([128, 128], BF, tag="m", bufs=2,
                                             name=f"tp{i}_{j}")
                            nc.tensor.transpose(
                                tp[:], vt[:, j * 128:(j + 1) * 128], ident[:])
                            nc.vector.tensor_copy(
                                vaug[:].rearrange(
                                    "p (j a c) -> p j a c", a=2, c=65)[
                                    :, jj, :, 0:64],
                                tp[:].rearrange("p (a c) -> p a c", c=64),
                            )
                        if j0 == 2:
                            vts.pop(i)
                    return emit

                qk = [((i, 'qk'), g_granule(g, k0)) for g in range(2)
                      for k0 in range(0, 8, 2)]
                v = [((i, 'v'), g_granule(2, k0)) for k0 in range(0, 8, 2)]
                v += [((i, 'v'), t_granule(0)), ((i, 'v'), t_granule(2))]
                return qk + v

            filler = deque()

            def pop_fillers(k):
                for _ in range(k):
                    if filler:
                        filler.popleft()[1]()

            def drain_through(key):
                """Pop until no unit with the given key remains (they form a
                contiguous run in FIFO order)."""
                while any(u[0] == key for u in filler):
                    filler.popleft()[1]()

            # state deferred from the previous chunk
            pending = None  # (i, b, n, otps, pts)

            def finish_pending():
                """Tail AVs for the previous chunk (emitted directly);
                normalization + phase-4 are returned as filler units."""
                nonlocal pending
                if pending is None:
                    return []
                pi, pb, pn, potps, ppts = pending
                pending = None
                for j in range(max(pn - AV_DELAY, 0), pn):
                    kgj = pb * 16 + j
                    for h in range(2):
                        nc.tensor.matmul(
                            potps[h][:],
                            vaug[:, kgj * 130 + h * 65:kgj * 130 + h * 65 + 65],
                            ppts[j][:, h * TC:(h + 1) * TC],
                            start=(j == 0), stop=(j == pn - 1),
                            skip_group_check=True,
                        )
                state = {}

                def norm_unit():
                    # 1/rowsum, heads stacked into one [128, TC] bf16 tile
                    ot = sbpool.tile([128, TC], BF, tag="ot", bufs=2,
                                     name=f"ot{pi}")
                    for h in range(2):
                        rch = sbpool.tile([1, TC], FPR, tag=f"rc{h}", bufs=2,
                                          name=f"rc{h}_{pi}")
                        with nc.allow_low_precision(reason="sums f32r"):
                            nc.scalar.copy(rch[:], potps[h][64:65, :])
                        bchh = pspool.tile([64, TC], FP, tag="m", bufs=2,
                                           name=f"bch{h}_{pi}")
                        nc.tensor.matmul(bchh[:], onesr_sb[0:1, :], rch[:],
                                         start=True, stop=True)
                        bcsh = sbpool.tile([64, TC], FP, tag=f"bcs{h}",
                                           bufs=2, name=f"bcs{h}_{pi}")
                        nc.vector.reciprocal_approx_fast(out=bcsh[:],
                                                         in_=bchh[:])
                        nc.vector.tensor_mul(ot[h * 64:(h + 1) * 64, :],
                                             potps[h][0:64, :], bcsh[:])
                    state['ot'] = ot
                    state['ys'] = sbpool.tile([128, BT], BF, tag="ys",
                                              bufs=2, name=f"ys{pi}")

                def m_unit(m):
                    def emit():
                        yp = pspool.tile([128, TC], FP, tag="m", bufs=2,
                                         name=f"yp{pi}_{m}")
                        nc.tensor.matmul(
                            yp[:], wout_sb[:, m * 128:(m + 1) * 128],
                            state['ot'][:], start=True, stop=True)
                        ys = state['ys']
                        if m % 2 == 0:
                            nc.scalar.copy(
                                ys[:, m * TC:(m + 1) * TC], yp[:])
                        else:
                            nc.vector.tensor_copy(
                                ys[:, m * TC:(m + 1) * TC], yp[:])
                            nc.sync.dma_start(
                                out=yt[pi * 128:(pi + 1) * 128,
                                       (m - 1) * TC:(m + 1) * TC],
                                in_=ys[:, (m - 1) * TC:(m + 1) * TC])
                    return emit

                return [((pi, 'p4'), norm_unit)] + \
                    [((pi, 'p4'), m_unit(m)) for m in range(8)]

            for tcx in range(NTC):
                b, qc = divmod(tcx, 4)
                t0 = tcx * TC
                n = 4 * (qc + 1)

                if tcx == 0:
                    for _, u in make_phase1_units(0):
                        u()
                    filler.extend(make_phase1_units(1))
                else:
                    # drain so phase-1(tcx) Q/K (needed by every score
                    # matmul of this chunk) is fully emitted; V granules
                    # may lag into the loop
                    drain_through((tcx, 'qk'))
                    filler.extend(finish_pending())
                    if tcx + 1 < NTC:
                        filler.extend(make_phase1_units(tcx + 1))

                otps = (
                    pspool.tile([65, TC], FP, tag="av0", bufs=1,
                                name=f"otp0_{tcx}"),
                    pspool.tile([65, TC], FP, tag="av1", bufs=1,
                                name=f"otp1_{tcx}"),
                )
                pts = {}
                for kt in range(n):
                    kg = b * 16 + kt
                    s = pspool.tile([128, 2 * TC], FP, tag="s", bufs=2,
                                    name=f"s{tcx}_{kt}")
                    for h in range(2):
                        nc.tensor.matmul(
                            s[:, h * TC:(h + 1) * TC],
                            KT[h * 64:(h + 1) * 64, kg * 128:(kg + 1) * 128],
                            QT[h * 64:(h + 1) * 64, t0:t0 + TC],
                            start=True, stop=True,
                        )
                    # V/transpose granules of this chunk must land before
                    # the diagonal AV matmuls need vaug
                    if kt >= max(n - 6, 0):
                        drain_through((tcx, 'v'))
                    # adaptive filler drain across remaining k-slots
                    slots = n - kt
                    pop_fillers(min(-(-len(filler) // slots), 3)
                                if filler else 0)
                    pt = sbpool.tile([128, 2 * TC], BF, tag="pt", bufs=6,
                                     name=f"pt{tcx}_{kt}")
                    nc.scalar.activation(pt[:], s[:], ACT.Exp, scale=0.125)
                    if kt >= n - 4:
                        # multiplicative causal mask on the bf16 exp output:
                        # cheaper on DVE and off the scores-PSUM release path
                        v = kt - (n - 4)
                        nc.vector.tensor_mul(
                            pt[:], pt[:],
                            masks_sb[:, v * 1024:(v + 1) * 1024])
                    pts[kt] = pt
                    if kt >= AV_DELAY:
                        j = kt - AV_DELAY
                        kgj = b * 16 + j
                        for h in range(2):
                            nc.tensor.matmul(
                                otps[h][:],
                                vaug[:, kgj * 130 + h * 65:
                                     kgj * 130 + h * 65 + 65],
                                pts[j][:, h * TC:(h + 1) * TC],
                                start=(j == 0), stop=(j == n - 1),
                                skip_group_check=True,
                            )
                pending = (tcx, b, n, otps, pts)

            pop_fillers(len(filler))
            for _, u in finish_pending():
                u()
    nc.compile()
    return nc


def make_in_maps(x, w_qkv, b_qkv, w_out):
    x = np.ascontiguousarray(np.asarray(x, np.float32).reshape(BT, C))
    xT = np.ascontiguousarray(x.T).astype(ml_dtypes.bfloat16)
    w_qkv = np.asarray(w_qkv, np.float32)
    b_qkv = np.asarray(b_qkv, np.float32)
    w_out = np.asarray(w_out, np.float32)

    # masks[p, v*1024 + h*512 + q] = 1 if (v*128 + p) <= q else 0
    # (multiplies the exp output: exact causal zeroing)
    kk = np.arange(128)[:, None, None, None] + 128 * np.arange(4)[None, :, None, None]
    qq = np.arange(512)[None, None, None, :]
    mask = np.where(kk <= qq, 1.0, 0.0) * np.ones((1, 1, 2, 1))
    mask = np.ascontiguousarray(
        mask.reshape(128, 4096)).astype(ml_dtypes.bfloat16)

    # wc[p, k*384 + g*128 + j] = w_qkv[k*128 + p, g*1024 + c0 + j]
    w4 = w_qkv.reshape(8, 128, 3, 1024)

    in_maps = []
    for c in range(NCORES):
        c0 = c * 128
        wcs = np.ascontiguousarray(
            w4[:, :, :, c0:c0 + 128].transpose(1, 0, 2, 3).reshape(128, 3072)
        ).astype(ml_dtypes.bfloat16)
        bq = np.ascontiguousarray(
            b_qkv.reshape(3, 1024)[:, c0:c0 + 128].T)
        in_maps.append({
            "xt": xT,
            "wc": wcs,
            "wout": np.ascontiguousarray(
                w_out[c0:c0 + 128, :]).astype(ml_dtypes.bfloat16),
            "bqkv": bq,
            "masks": mask,
            "onesr": np.ones((1, 64), np.float32),
        })
    return in_maps


_NC_CACHE = None


def kernel(x, w_qkv, b_qkv, w_out, b_out):
    global _NC_CACHE, LAST_RESULTS
    if _NC_CACHE is None:
        _NC_CACHE = build_nc()
    nc = _NC_CACHE

    in_maps = make_in_maps(x, w_qkv, b_qkv, w_out)

    res = run_bass_kernel_spmd(
        nc, in_maps, list(range(NCORES)),
        trace=bool(os.environ.get("BASS_TRACE")),
    )
    LAST_RESULTS = res

    acc = np.zeros((C, BT), np.float64)
    for out_map in res.results:
        # yt[i*128 + p, m*512 + t] -> y_partial[m*128 + p, i*512 + t]
        yp = out_map["yt"].astype(np.float32)
        yp = yp.reshape(8, 128, 8, 512).transpose(2, 1, 0, 3).reshape(C, BT)
        acc += yp
    y = acc.T.astype(np.float32) + np.asarray(b_out, np.float32)[None, :]
    return y.reshape(B, T, C)
